# revision 39
# baseline (speedup 1.0000x reference)
"""Trainium2 Bass kernel for causal multi-head attention.

Problem: x[2, 2048, 1024], W_Q/W_K/W_V/W_O [1024, 1024], 16 heads, d_k=64,
causal softmax attention, fp32.

Sharding (8 cores): core c owns batch b=c//4 and head-group g=c%4 (4 heads,
256 cols of W_Q/K/V, 256 rows of W_O). Each core computes a full [S, D]
partial output (its 4 heads' contribution through W_O) in fp16; host sums
the 4 partials per batch in fp32.

Design notes (v3):
  - All PE inputs fp16: LDWEIGHTS ~100ns, fully hidden under the 213ns
    512-wide matmul stream (measured 216ns/MM warm, at roofline).
  - Scores MMs for the two heads of a pair interleaved (h0kc0, h1kc0,
    h0kc1, h1kc1): dk=64 contraction -> row groups (0,0)/(64,0) run
    concurrently on the PE.
  - NT stored pair-stacked [128, S]; W_O matmuls contract over 128 (2 heads
    at once) and accumulate both pairs into one PSUM tile -> single fp16
    output tensor.
  - Exp causal-trimmed; tri-mask muls on GpSimd (SBUF-only) to keep the
    DVE queue short; a->os casts on ScalarE; batched single-issue DMAs.
  - Schedule: per q-tile, pair-0 round then pair-1 round; projections for
    the NEXT half-round and W_O for the previous q-tile are issued as PE
    fillers between score groups so no engine queue ever gates the PE at a
    round boundary (HAM stays warm).
"""

import numpy as np
from contextlib import ExitStack

import concourse.bass as bass
import concourse.tile as tile
from concourse import bacc, mybir
from concourse.bass_utils import run_bass_kernel_spmd

dt = mybir.dt
AF = mybir.ActivationFunctionType

B, S, D, NH, DK = 2, 2048, 1024, 16, 64
NCORES = 8
HPC = 4            # heads per core
CW = HPC * DK      # 256 per-core col width of W_Q/K/V (rows of W_O)
QT_W = 512         # q-tile width
KC_W = 128         # k-chunk width
NQT = S // QT_W    # 4
NKC = S // KC_W    # 16
NDC = D // 128     # 8 contraction chunks for projections
VPW = DK + 1       # 65: V chunk + ones column


def build(debug=False):
    nc = bacc.Bacc("TRN2", target_bir_lowering=False, debug=False,
                   num_devices=NCORES)

    # Inputs are pre-transposed host-side so every DMA line is contiguous
    # per partition (strided DRAM reads showed ~8x read amplification).
    xt_d = nc.dram_tensor("xt", [NQT, 128, NDC, QT_W], dt.float16,
                          kind="ExternalInput").ap()
    wq_d = nc.dram_tensor("wq", [128, NDC, CW], dt.float16,
                          kind="ExternalInput").ap()
    wk_d = nc.dram_tensor("wk", [128, NDC, CW], dt.float16,
                          kind="ExternalInput").ap()
    wv_d = nc.dram_tensor("wv", [128, NDC, CW], dt.float16,
                          kind="ExternalInput").ap()
    wo_d = nc.dram_tensor("wo", [128, 2, D], dt.float16,
                          kind="ExternalInput").ap()
    on_d = nc.dram_tensor("ones", [DK + 1, DK], dt.float16,
                          kind="ExternalInput").ap()
    tri_d = nc.dram_tensor("tri", [KC_W, KC_W], dt.float16,
                           kind="ExternalInput").ap()
    o_d = nc.dram_tensor("o", [D, S], dt.float16, kind="ExternalOutput").ap()
    # pair-0's W_O contribution for the last q-tile (host adds it): lets
    # half of the final W_O stage run before the last norm completes
    o2_d = nc.dram_tensor("o2", [D, QT_W], dt.float16,
                          kind="ExternalOutput").ap()
    dbg = {}
    if debug:
        for nm, shp, dty in (("dbg_qt", [128, S], dt.float16),
                             ("dbg_kt", [128, S], dt.float16),
                             ("dbg_vp", [128, HPC * NKC * VPW], dt.float16),
                             ("dbg_os", [DK + 1, S], dt.float16),
                             ("dbg_nt", [128, S], dt.float16)):
            dbg[nm] = nc.dram_tensor(nm, shp, dty, kind="ExternalOutput").ap()

    ts = bass.ts

    with tile.TileContext(nc) as tc, ExitStack() as top:
        p_const = top.enter_context(tc.tile_pool(name="const", bufs=2))
        p_w = top.enter_context(tc.tile_pool(name="w", bufs=3))
        p_wo = top.enter_context(tc.tile_pool(name="wo", bufs=1))
        p_xt = top.enter_context(tc.tile_pool(name="xt", bufs=1))
        p_qt = top.enter_context(tc.tile_pool(name="qt", bufs=2))
        p_kt = top.enter_context(tc.tile_pool(name="kt", bufs=2))
        p_vp = top.enter_context(tc.tile_pool(name="vp", bufs=1))
        p_nt = top.enter_context(tc.tile_pool(name="nt", bufs=2))
        p_os = top.enter_context(tc.tile_pool(name="os", bufs=HPC))
        p_e = top.enter_context(tc.tile_pool(name="e", bufs=6))
        p_rc = top.enter_context(tc.tile_pool(name="rc", bufs=4))
        p_rh = top.enter_context(tc.tile_pool(name="rh", bufs=4))
        p_oc = top.enter_context(tc.tile_pool(name="oc", bufs=4))
        p_s = top.enter_context(tc.tile_pool(name="s", bufs=2, space="PSUM"))
        p_a = top.enter_context(tc.tile_pool(name="a", bufs=2, space="PSUM"))
        p_pt = top.enter_context(tc.tile_pool(name="pt", bufs=2, space="PSUM"))

        # ---- V-natural tile; memset its ones-columns first on gpsimd ----
        vp_sb = p_vp.tile([128, HPC, NKC, VPW], dt.float16, name="vp",
                          tag="vp")
        nc.gpsimd.memset(vp_sb[:, :, :, DK:DK + 1], 1.0)

        # ---- batched weight/const DMAs (gpsimd queue, ~1 issue each) ----
        ones = p_const.tile([DK + 1, DK], dt.float16, name="ones", tag="ones")
        tri = p_const.tile([KC_W, KC_W], dt.float16, name="tri", tag="tri")
        w_sb = {m: p_w.tile([128, NDC, CW], dt.float16, name=f"w{m}",
                            tag="w")
                for m in ("k", "q", "v")}
        wo_sb = p_wo.tile([128, 2, D], dt.float16, name="wo", tag="wo")
        # wk gates the first matmul: split it across two queues
        nc.gpsimd.dma_start(out=w_sb["k"][:, 0:4], in_=wk_d[:, 0:4])
        nc.scalar.dma_start(out=w_sb["k"][:, 4:8], in_=wk_d[:, 4:8])
        nc.gpsimd.dma_start(out=w_sb["q"][:], in_=wq_d[:])
        nc.gpsimd.dma_start(out=tri[:], in_=tri_d[:])
        nc.gpsimd.dma_start(out=w_sb["v"][:], in_=wv_d[:])
        nc.gpsimd.dma_start(out=wo_sb[:], in_=wo_d[:])
        nc.gpsimd.dma_start(out=ones[:], in_=on_d[:])

        # ---- x^T: first block split across sync+scalar queues (it gates
        # the first matmul); st1 on scalar, rest on sync ----
        xt_sb = p_xt.tile([128, NDC, S], dt.float16, name="xt", tag="xt")
        nc.sync.dma_start(out=xt_sb[:, 0:4, 0:QT_W], in_=xt_d[0][:, 0:4])
        nc.scalar.dma_start(out=xt_sb[:, 4:8, 0:QT_W], in_=xt_d[0][:, 4:8])
        for st in range(1, NQT):
            eng = nc.scalar if st == 1 else nc.sync
            eng.dma_start(out=xt_sb[:, :, ts(st, QT_W)], in_=xt_d[st])

        # ---- persistent tiles ----
        qt_sb = [p_qt.tile([128, S], dt.float16, name="qt", tag="qt")
                 for _ in range(2)]
        kt_sb = [p_kt.tile([128, S], dt.float16, name="kt", tag="kt")
                 for _ in range(2)]
        nt_sb = [p_nt.tile([128, S], dt.float16, name="nt", tag="nt")
                 for _ in range(2)]
        os_sb = [p_os.tile([DK + 1, S], dt.float16, name="os", tag="os")
                 for _ in range(HPC)]

        # ================= pipeline units =================

        def proj(mat, st, pg):
            """(x @ W)^T chunk -> qt/kt_sb[pg][:, st*512:]."""
            pp = p_pt.tile([128, QT_W], dt.float32, name="pp", tag="pt")
            for dc in range(NDC):
                nc.tensor.matmul(
                    pp[:],
                    w_sb[mat][:, dc, ts(pg, 128)],
                    xt_sb[:, dc, ts(st, QT_W)],
                    start=(dc == 0), stop=(dc == NDC - 1),
                )
            dst = (qt_sb if mat == "q" else kt_sb)[pg][:, ts(st, QT_W)]
            nc.vector.tensor_copy(dst, pp[:])

        def vnat(sc):
            """V rows [128*sc, 128*sc+128) for all 4 heads, natural layout."""
            pv = p_pt.tile([128, CW], dt.float32, name="pv", tag="pt")
            for dc in range(NDC):
                nc.tensor.matmul(
                    pv[:],
                    xt_sb[:, dc, ts(sc, KC_W)],
                    w_sb["v"][:, dc, :],
                    start=(dc == 0), stop=(dc == NDC - 1),
                )
            nc.vector.tensor_copy(
                vp_sb[:, :, sc, 0:DK],
                pv.rearrange("p (h d) -> p h d", h=HPC),
            )

        def _pt_tile(pool):
            """W_O psum tile; from p_pt, or carved out of a scores-pool
            buffer (same name/tag/shape so the pool doesn't grow)."""
            if pool is None:
                return p_pt.tile([128, QT_W], dt.float32, name="pt",
                                 tag="pt")
            s = pool.tile([128, 2 * QT_W], dt.float32, name="s", tag="s")
            return s[:, 0:QT_W]

        def wo_unit(ec, st, pool=None):
            """o^T[ec*128:, st*512:] = sum over both head pairs."""
            pt = _pt_tile(pool)
            for hp in range(2):
                nc.tensor.matmul(
                    pt[:],
                    wo_sb[:, hp, ts(ec, 128)],
                    nt_sb[hp][:, ts(st, QT_W)],
                    start=(hp == 0), stop=(hp == 1),
                )
            oc = p_oc.tile([128, QT_W], dt.float16, name="oc", tag="oc")
            nc.vector.tensor_copy(oc[:], pt[:])
            nc.sync.dma_start(out=o_d[ts(ec, 128), ts(st, QT_W)], in_=oc[:])

        def wo_half(ec, st, hp, cast_eng=None, pool=None):
            """Single-pair W_O contribution; hp=0 -> o2, hp=1 -> o."""
            pt = _pt_tile(pool)
            nc.tensor.matmul(
                pt[:],
                wo_sb[:, hp, ts(ec, 128)],
                nt_sb[hp][:, ts(st, QT_W)],
                start=True, stop=True,
            )
            oc = p_oc.tile([128, QT_W], dt.float16, name="oc", tag="oc")
            if cast_eng is nc.scalar:
                nc.scalar.copy(oc[:], pt[:])
            else:
                nc.vector.tensor_copy(oc[:], pt[:])
            if hp == 0:
                nc.sync.dma_start(out=o2_d[ts(ec, 128), :], in_=oc[:])
            else:
                nc.sync.dma_start(out=o_d[ts(ec, 128), ts(st, QT_W)],
                                  in_=oc[:])

        def scores_exp(p, qt, g):
            """Both heads of pair p, kcs (2g, 2g+1): scores + exp + mask."""
            kcs = (2 * g, 2 * g + 1)
            s2 = [p_s.tile([128, 2 * QT_W], dt.float32, name="s", tag="s")
                  for _ in range(2)]
            e2 = [p_e.tile([128, 2 * QT_W], dt.float16, name="e", tag="e")
                  for _ in range(2)]
            for j, kc in enumerate(kcs):
                for e in range(2):  # head within pair; interleave row groups
                    prow = slice(e * DK, (e + 1) * DK)
                    nc.tensor.matmul(
                        s2[e][:, ts(j, QT_W)],
                        kt_sb[p][prow, ts(kc, KC_W)],
                        qt_sb[p][prow, ts(qt, QT_W)],
                        start=True, stop=True,
                    )
            los = [max(0, (kc - 4 * qt)) * KC_W for kc in kcs]
            for e in range(2):
                if los == [0, 0]:
                    nc.scalar.activation(e2[e][:], s2[e][:], AF.Exp,
                                         scale=0.125)
                else:
                    for j in range(2):
                        sl = slice(j * QT_W + los[j], (j + 1) * QT_W)
                        nc.scalar.activation(e2[e][:, sl], s2[e][:, sl],
                                             AF.Exp, scale=0.125)
                for j, kc in enumerate(kcs):
                    r = kc - 4 * qt
                    if 0 <= r:
                        lo = j * QT_W + r * KC_W
                        nc.vector.tensor_mul(
                            e2[e][:, lo:lo + KC_W],
                            e2[e][:, lo:lo + KC_W],
                            tri[:],
                        )
            return e2

        def attnv_head(p, qt, g, e, eh, ah, nkc):
            h = 2 * p + e
            for j, kc in enumerate((2 * g, 2 * g + 1)):
                r = kc - 4 * qt
                vsl = vp_sb[:, h, kc, :]
                if r > 0:
                    lo = r * KC_W
                    nc.tensor.matmul(
                        ah[:, lo:QT_W],
                        vsl,
                        eh[:, j * QT_W + lo:(j + 1) * QT_W],
                        start=False, stop=(kc == nkc - 1),
                    )
                else:
                    nc.tensor.matmul(
                        ah[:],
                        vsl,
                        eh[:, ts(j, QT_W)],
                        start=(kc == 0), stop=(kc == nkc - 1),
                    )

        def attnv(p, qt, g, e2, a2, nkc):
            for e in range(2):
                attnv_head(p, qt, g, e, e2[e], a2[e], nkc)

        def attn_round(p, qt, fillers):
            """Pair p, q-tile qt. fillers: list of closures to interleave."""
            nkc = 4 * (qt + 1)
            ngr = nkc // 2
            nfill = len(fillers)
            a2 = [p_a.tile([DK + 1, QT_W], dt.float32, name="a", tag="a")
                  for _ in range(2)]
            prev = None
            fi = 0
            for g in range(ngr):
                e2 = scores_exp(p, qt, g)
                upto = (nfill * (g + 1)) // ngr
                while fi < upto:
                    fillers[fi]()
                    fi += 1
                if prev is not None:
                    attnv(p, qt, g - 1, prev, a2, nkc)
                prev = e2
            rhs = []
            for e in range(2):
                # finish each head's attnv then immediately start its
                # recip/rh chain — shortens the tail's critical path
                attnv_head(p, qt, ngr - 1, e, prev[e], a2[e], nkc)
                rc = p_rc.tile([DK + 1, QT_W], dt.float32, name="rc", tag="rc")
                nc.vector.reciprocal_approx_fast(out=rc[:], in_=a2[e][:])
                nc.scalar.copy(os_sb[2 * p + e][:, ts(qt, QT_W)], a2[e][:])
                rh = p_rh.tile([DK + 1, QT_W], dt.float16, name="rh", tag="rh")
                nc.vector.tensor_copy(rh[DK:DK + 1, :], rc[DK:DK + 1, :])
                rhs.append(rh)
            return rhs

        def norm(h, qt, rh):
            """nt_pair[h] [:, qt] = os[h] * (1/denom) broadcast."""
            p, e = h // 2, h % 2
            bc = p_pt.tile([DK, QT_W], dt.float32, name="bc", tag="pt")
            nc.tensor.matmul(
                bc[:],
                ones[DK:DK + 1, :],
                rh[DK:DK + 1, :],
                start=True, stop=True,
            )
            nc.vector.tensor_mul(
                nt_sb[p][e * DK:(e + 1) * DK, ts(qt, QT_W)],
                os_sb[h][0:DK, ts(qt, QT_W)],
                bc[:],
            )

        # ================= schedule =================
        def F(*cs):
            return list(cs)

        def vnat_f(lo):
            return [lambda sc=sc: vnat(sc) for sc in range(lo, lo + 4)]

        def proj_f(st, pg):
            return [lambda: proj("k", st, pg), lambda: proj("q", st, pg)]

        def wo_f(st):
            return [lambda ec=ec: wo_unit(ec, st) for ec in range(NDC)]

        def norm_f(qt, rhs2, p):
            return [lambda e=e: norm(2 * p + e, qt, rhs2[e]) for e in (0, 1)]

        proj("k", 0, 0)
        proj("q", 0, 0)
        # W_O fillers are biased toward the late rounds: qt=3's exp load
        # saturates ScalarE, so the PE needs the most pad work there.
        rhs1 = None
        for qt in range(NQT):
            f0 = F(*proj_f(qt, 1), *vnat_f(4 * qt))
            if qt > 0:
                f0 += norm_f(qt - 1, rhs1, 1)
            rhs0 = attn_round(0, qt, f0)
            f1 = F(*(proj_f(qt + 1, 0) if qt < NQT - 1 else []))
            f1 += norm_f(qt, rhs0, 0)
            if qt == 1:
                f1 += wo_f(0)
            elif qt == 2:
                f1 += wo_f(1)[:5]
            elif qt == 3:
                # W_O fillers with no new dependencies come FIRST (qt=3 has
                # no proj fillers, and norm's bc would stall the PE queue on
                # the pair-0 recip/rh chain); then norm(3,p0), then pair-0's
                # half of W_O(3) which needs it.
                f1 = F(*wo_f(1)[5:], *norm_f(qt, rhs0, 0),
                       *(lambda ec=ec: wo_half(ec, 3, 0)
                         for ec in range(NDC)),
                       *wo_f(2)[:4])
            rhs1 = attn_round(1, qt, f1)
        # tail: reserved W_O(2) units pad the PE while the pair-1 recip/rh
        # chain for qt=3 drains. The scores PSUM pool is free now, so tail
        # units draw from it — their matmuls never wait on the backed-up
        # DVE queue via p_pt rotation. Casts alternate DVE/ScalarE.
        for i, ec in enumerate(range(4, NDC)):
            wo_unit(ec, NQT - 2, pool=p_s if i % 2 else None)
        for f in norm_f(NQT - 1, rhs1, 1):
            f()
        for ec in range(NDC):
            wo_half(ec, 3, 1, cast_eng=nc.scalar if ec % 2 else nc.vector,
                    pool=p_s if ec % 2 else None)

        if debug:
            nc.sync.dma_start(out=dbg["dbg_qt"][:], in_=qt_sb[0][:])
            nc.sync.dma_start(out=dbg["dbg_kt"][:], in_=kt_sb[0][:])
            nc.sync.dma_start(
                out=dbg["dbg_vp"][:],
                in_=vp_sb.rearrange("p h k w -> p (h k w)"))
            nc.sync.dma_start(out=dbg["dbg_os"][:], in_=os_sb[0][:])
            nc.sync.dma_start(out=dbg["dbg_nt"][:], in_=nt_sb[0][:])

    nc.compile()
    return nc


_NC = None


def _get_nc():
    global _NC
    if _NC is None:
        _NC = build()
    return _NC


def make_in_maps(x, W_Q, W_K, W_V, W_O):
    x = np.asarray(x, np.float32)
    W_Q, W_K, W_V, W_O = (np.asarray(w, np.float32)
                          for w in (W_Q, W_K, W_V, W_O))
    ones = np.ones((DK + 1, DK), np.float16)
    trim = (np.arange(KC_W)[:, None] <= np.arange(KC_W)[None, :]).astype(
        np.float16)
    def wlay(w):
        # [D, CW] -> [128, NDC, CW]: partition-contiguous DMA lines
        return np.ascontiguousarray(
            w.reshape(NDC, 128, CW).transpose(1, 0, 2)).astype(np.float16)

    in_maps = []
    for c in range(NCORES):
        b, g = c // HPC, c % HPC
        cols = slice(g * CW, (g + 1) * CW)
        # x[b].T is [D, S]; target [st][p][dc][j] with d = dc*128+p,
        # s = st*512+j
        xt = np.ascontiguousarray(
            x[b].T.reshape(NDC, 128, NQT, QT_W).transpose(2, 1, 0, 3)
        ).astype(np.float16)
        wo = np.ascontiguousarray(
            W_O[cols, :].reshape(2, 128, D).transpose(1, 0, 2)
        ).astype(np.float16)
        in_maps.append({
            "xt": xt,
            "wq": wlay(W_Q[:, cols]),
            "wk": wlay(W_K[:, cols]),
            "wv": wlay(W_V[:, cols]),
            "wo": wo,
            "ones": ones,
            "tri": trim,
        })
    return in_maps


def gather_output(results):
    out = np.zeros((B, S, D), np.float32)
    for c in range(NCORES):
        o = results[c]["o"].astype(np.float32)
        o[:, (NQT - 1) * QT_W:] += results[c]["o2"].astype(np.float32)
        out[c // HPC] += o.T
    return out


def kernel(x, W_Q, W_K, W_V, W_O):
    nc = _get_nc()
    res = run_bass_kernel_spmd(
        nc, make_in_maps(x, W_Q, W_K, W_V, W_O), list(range(NCORES))).results
    return gather_output(res)


# revision 41
# speedup vs baseline: 1.0461x; 1.0461x over previous
"""Trainium2 Bass kernel for causal multi-head attention.

Problem: x[2, 2048, 1024], W_Q/W_K/W_V/W_O [1024, 1024], 16 heads, d_k=64,
causal softmax attention, fp32.

Sharding (8 cores): core c owns batch b=c//4 and head-group g=c%4 (4 heads,
256 cols of W_Q/K/V, 256 rows of W_O). Each core computes a full [S, D]
partial output (its 4 heads' contribution through W_O) in fp16; host sums
the 4 partials per batch in fp32.

Design notes (v3):
  - All PE inputs fp16: LDWEIGHTS ~100ns, fully hidden under the 213ns
    512-wide matmul stream (measured 216ns/MM warm, at roofline).
  - Scores MMs for the two heads of a pair interleaved (h0kc0, h1kc0,
    h0kc1, h1kc1): dk=64 contraction -> row groups (0,0)/(64,0) run
    concurrently on the PE.
  - NT stored pair-stacked [128, S]; W_O matmuls contract over 128 (2 heads
    at once) and accumulate both pairs into one PSUM tile -> single fp16
    output tensor.
  - Exp causal-trimmed; tri-mask muls on GpSimd (SBUF-only) to keep the
    DVE queue short; a->os casts on ScalarE; batched single-issue DMAs.
  - Schedule: per q-tile, pair-0 round then pair-1 round; projections for
    the NEXT half-round and W_O for the previous q-tile are issued as PE
    fillers between score groups so no engine queue ever gates the PE at a
    round boundary (HAM stays warm).
"""

import numpy as np
from contextlib import ExitStack

import concourse.bass as bass
import concourse.tile as tile
from concourse import bacc, mybir
from concourse.bass_utils import run_bass_kernel_spmd

dt = mybir.dt
AF = mybir.ActivationFunctionType

B, S, D, NH, DK = 2, 2048, 1024, 16, 64
NCORES = 8
HPC = 4            # heads per core
CW = HPC * DK      # 256 per-core col width of W_Q/K/V (rows of W_O)
QT_W = 512         # q-tile width
KC_W = 128         # k-chunk width
NQT = S // QT_W    # 4
NKC = S // KC_W    # 16
NDC = D // 128     # 8 contraction chunks for projections
VPW = DK + 1       # 65: V chunk + ones column


def build(debug=False):
    nc = bacc.Bacc("TRN2", target_bir_lowering=False, debug=False,
                   num_devices=NCORES)

    # Inputs are pre-transposed host-side so every DMA line is contiguous
    # per partition (strided DRAM reads showed ~8x read amplification).
    xt_d = nc.dram_tensor("xt", [NQT, 128, NDC, QT_W], dt.float16,
                          kind="ExternalInput").ap()
    wq_d = nc.dram_tensor("wq", [128, NDC, CW], dt.float16,
                          kind="ExternalInput").ap()
    wk_d = nc.dram_tensor("wk", [128, NDC, CW], dt.float16,
                          kind="ExternalInput").ap()
    wv_d = nc.dram_tensor("wv", [128, NDC, CW], dt.float16,
                          kind="ExternalInput").ap()
    wo_d = nc.dram_tensor("wo", [128, 2, D], dt.float16,
                          kind="ExternalInput").ap()
    on_d = nc.dram_tensor("ones", [DK + 1, DK], dt.float16,
                          kind="ExternalInput").ap()
    tri_d = nc.dram_tensor("tri", [KC_W, KC_W], dt.float16,
                           kind="ExternalInput").ap()
    o_d = nc.dram_tensor("o", [D, S], dt.float16, kind="ExternalOutput").ap()
    # pair-0's W_O contribution for the last q-tile (host adds it): lets
    # half of the final W_O stage run before the last norm completes
    o2_d = nc.dram_tensor("o2", [D, QT_W], dt.float16,
                          kind="ExternalOutput").ap()
    dbg = {}
    if debug:
        for nm, shp, dty in (("dbg_qt", [128, S], dt.float16),
                             ("dbg_kt", [128, S], dt.float16),
                             ("dbg_vp", [128, HPC * NKC * VPW], dt.float16),
                             ("dbg_os", [DK + 1, S], dt.float16),
                             ("dbg_nt", [128, S], dt.float16)):
            dbg[nm] = nc.dram_tensor(nm, shp, dty, kind="ExternalOutput").ap()

    ts = bass.ts

    with tile.TileContext(nc) as tc, ExitStack() as top:
        p_const = top.enter_context(tc.tile_pool(name="const", bufs=2))
        p_w = top.enter_context(tc.tile_pool(name="w", bufs=3))
        p_wo = top.enter_context(tc.tile_pool(name="wo", bufs=1))
        p_xt = top.enter_context(tc.tile_pool(name="xt", bufs=1))
        p_qt = top.enter_context(tc.tile_pool(name="qt", bufs=2))
        p_kt = top.enter_context(tc.tile_pool(name="kt", bufs=2))
        p_vp = top.enter_context(tc.tile_pool(name="vp", bufs=1))
        p_nt = top.enter_context(tc.tile_pool(name="nt", bufs=2))
        p_os = top.enter_context(tc.tile_pool(name="os", bufs=HPC))
        p_e = top.enter_context(tc.tile_pool(name="e", bufs=6))
        p_rc = top.enter_context(tc.tile_pool(name="rc", bufs=4))
        p_rh = top.enter_context(tc.tile_pool(name="rh", bufs=4))
        p_oc = top.enter_context(tc.tile_pool(name="oc", bufs=4))
        p_s = top.enter_context(tc.tile_pool(name="s", bufs=2, space="PSUM"))
        p_a = top.enter_context(tc.tile_pool(name="a", bufs=2, space="PSUM"))
        p_pt = top.enter_context(tc.tile_pool(name="pt", bufs=2, space="PSUM"))

        # ---- V-natural tile; memset its ones-columns first on gpsimd ----
        vp_sb = p_vp.tile([128, HPC, NKC, VPW], dt.float16, name="vp",
                          tag="vp")
        nc.gpsimd.memset(vp_sb[:, :, :, DK:DK + 1], 1.0)

        # ---- batched weight/const DMAs (gpsimd queue, ~1 issue each) ----
        ones = p_const.tile([DK + 1, DK], dt.float16, name="ones", tag="ones")
        tri = p_const.tile([KC_W, KC_W], dt.float16, name="tri", tag="tri")
        w_sb = {m: p_w.tile([128, NDC, CW], dt.float16, name=f"w{m}",
                            tag="w")
                for m in ("k", "q", "v")}
        wo_sb = p_wo.tile([128, 2, D], dt.float16, name="wo", tag="wo")
        nc.gpsimd.dma_start(out=w_sb["k"][:], in_=wk_d[:])
        nc.gpsimd.dma_start(out=w_sb["q"][:], in_=wq_d[:])
        nc.gpsimd.dma_start(out=tri[:], in_=tri_d[:])
        nc.gpsimd.dma_start(out=w_sb["v"][:], in_=wv_d[:])
        nc.gpsimd.dma_start(out=wo_sb[:], in_=wo_d[:])
        nc.gpsimd.dma_start(out=ones[:], in_=on_d[:])

        # ---- x^T: first block split across sync+scalar queues (it gates
        # the first matmul); st1 on scalar, rest on sync ----
        xt_sb = p_xt.tile([128, NDC, S], dt.float16, name="xt", tag="xt")
        nc.sync.dma_start(out=xt_sb[:, 0:4, 0:QT_W], in_=xt_d[0][:, 0:4])
        nc.scalar.dma_start(out=xt_sb[:, 4:8, 0:QT_W], in_=xt_d[0][:, 4:8])
        for st in range(1, NQT):
            eng = nc.scalar if st == 1 else nc.sync
            eng.dma_start(out=xt_sb[:, :, ts(st, QT_W)], in_=xt_d[st])

        # ---- persistent tiles ----
        qt_sb = [p_qt.tile([128, S], dt.float16, name="qt", tag="qt")
                 for _ in range(2)]
        kt_sb = [p_kt.tile([128, S], dt.float16, name="kt", tag="kt")
                 for _ in range(2)]
        nt_sb = [p_nt.tile([128, S], dt.float16, name="nt", tag="nt")
                 for _ in range(2)]
        os_sb = [p_os.tile([DK + 1, S], dt.float16, name="os", tag="os")
                 for _ in range(HPC)]

        # ================= pipeline units =================

        def proj(mat, st, pg):
            """(x @ W)^T chunk -> qt/kt_sb[pg][:, st*512:]."""
            pp = p_pt.tile([128, QT_W], dt.float32, name="pp", tag="pt")
            for dc in range(NDC):
                nc.tensor.matmul(
                    pp[:],
                    w_sb[mat][:, dc, ts(pg, 128)],
                    xt_sb[:, dc, ts(st, QT_W)],
                    start=(dc == 0), stop=(dc == NDC - 1),
                )
            dst = (qt_sb if mat == "q" else kt_sb)[pg][:, ts(st, QT_W)]
            nc.vector.tensor_copy(dst, pp[:])

        def vnat(sc):
            """V rows [128*sc, 128*sc+128) for all 4 heads, natural layout."""
            pv = p_pt.tile([128, CW], dt.float32, name="pv", tag="pt")
            for dc in range(NDC):
                nc.tensor.matmul(
                    pv[:],
                    xt_sb[:, dc, ts(sc, KC_W)],
                    w_sb["v"][:, dc, :],
                    start=(dc == 0), stop=(dc == NDC - 1),
                )
            nc.vector.tensor_copy(
                vp_sb[:, :, sc, 0:DK],
                pv.rearrange("p (h d) -> p h d", h=HPC),
            )

        def _pt_tile(pool):
            """W_O psum tile; from p_pt, or carved out of a scores-pool
            buffer (same name/tag/shape so the pool doesn't grow)."""
            if pool is None:
                return p_pt.tile([128, QT_W], dt.float32, name="pt",
                                 tag="pt")
            s = pool.tile([128, 2 * QT_W], dt.float32, name="s", tag="s")
            return s[:, 0:QT_W]

        def wo_unit(ec, st, pool=None):
            """o^T[ec*128:, st*512:] = sum over both head pairs."""
            pt = _pt_tile(pool)
            for hp in range(2):
                nc.tensor.matmul(
                    pt[:],
                    wo_sb[:, hp, ts(ec, 128)],
                    nt_sb[hp][:, ts(st, QT_W)],
                    start=(hp == 0), stop=(hp == 1),
                )
            oc = p_oc.tile([128, QT_W], dt.float16, name="oc", tag="oc")
            nc.vector.tensor_copy(oc[:], pt[:])
            nc.sync.dma_start(out=o_d[ts(ec, 128), ts(st, QT_W)], in_=oc[:])

        def wo_half(ec, st, hp, cast_eng=None, pool=None):
            """Single-pair W_O contribution; hp=0 -> o2, hp=1 -> o."""
            pt = _pt_tile(pool)
            nc.tensor.matmul(
                pt[:],
                wo_sb[:, hp, ts(ec, 128)],
                nt_sb[hp][:, ts(st, QT_W)],
                start=True, stop=True,
            )
            oc = p_oc.tile([128, QT_W], dt.float16, name="oc", tag="oc")
            if cast_eng is nc.scalar:
                nc.scalar.copy(oc[:], pt[:])
            else:
                nc.vector.tensor_copy(oc[:], pt[:])
            if hp == 0:
                nc.sync.dma_start(out=o2_d[ts(ec, 128), :], in_=oc[:])
            else:
                nc.sync.dma_start(out=o_d[ts(ec, 128), ts(st, QT_W)],
                                  in_=oc[:])

        def scores_exp(p, qt, g):
            """Both heads of pair p, kcs (2g, 2g+1): scores + exp + mask."""
            kcs = (2 * g, 2 * g + 1)
            s2 = [p_s.tile([128, 2 * QT_W], dt.float32, name="s", tag="s")
                  for _ in range(2)]
            e2 = [p_e.tile([128, 2 * QT_W], dt.float16, name="e", tag="e")
                  for _ in range(2)]
            for j, kc in enumerate(kcs):
                for e in range(2):  # head within pair; interleave row groups
                    prow = slice(e * DK, (e + 1) * DK)
                    nc.tensor.matmul(
                        s2[e][:, ts(j, QT_W)],
                        kt_sb[p][prow, ts(kc, KC_W)],
                        qt_sb[p][prow, ts(qt, QT_W)],
                        start=True, stop=True,
                    )
            los = [max(0, (kc - 4 * qt)) * KC_W for kc in kcs]
            for e in range(2):
                if los == [0, 0]:
                    nc.scalar.activation(e2[e][:], s2[e][:], AF.Exp,
                                         scale=0.125)
                else:
                    for j in range(2):
                        sl = slice(j * QT_W + los[j], (j + 1) * QT_W)
                        nc.scalar.activation(e2[e][:, sl], s2[e][:, sl],
                                             AF.Exp, scale=0.125)
                for j, kc in enumerate(kcs):
                    r = kc - 4 * qt
                    if 0 <= r:
                        lo = j * QT_W + r * KC_W
                        nc.vector.tensor_mul(
                            e2[e][:, lo:lo + KC_W],
                            e2[e][:, lo:lo + KC_W],
                            tri[:],
                        )
            return e2

        def attnv_head(p, qt, g, e, eh, ah, nkc):
            h = 2 * p + e
            for j, kc in enumerate((2 * g, 2 * g + 1)):
                r = kc - 4 * qt
                vsl = vp_sb[:, h, kc, :]
                if r > 0:
                    lo = r * KC_W
                    nc.tensor.matmul(
                        ah[:, lo:QT_W],
                        vsl,
                        eh[:, j * QT_W + lo:(j + 1) * QT_W],
                        start=False, stop=(kc == nkc - 1),
                    )
                else:
                    nc.tensor.matmul(
                        ah[:],
                        vsl,
                        eh[:, ts(j, QT_W)],
                        start=(kc == 0), stop=(kc == nkc - 1),
                    )

        def attnv(p, qt, g, e2, a2, nkc):
            for e in range(2):
                attnv_head(p, qt, g, e, e2[e], a2[e], nkc)

        def attn_round(p, qt, fillers):
            """Pair p, q-tile qt. fillers: list of closures to interleave."""
            nkc = 4 * (qt + 1)
            ngr = nkc // 2
            nfill = len(fillers)
            a2 = [p_a.tile([DK + 1, QT_W], dt.float32, name="a", tag="a")
                  for _ in range(2)]
            prev = None
            fi = 0
            for g in range(ngr):
                e2 = scores_exp(p, qt, g)
                upto = (nfill * (g + 1)) // ngr
                while fi < upto:
                    fillers[fi]()
                    fi += 1
                if prev is not None:
                    attnv(p, qt, g - 1, prev, a2, nkc)
                prev = e2
            attnv(p, qt, ngr - 1, prev, a2, nkc)
            rhs = []
            for e in range(2):
                rc = p_rc.tile([DK + 1, QT_W], dt.float32, name="rc", tag="rc")
                nc.vector.reciprocal_approx_fast(out=rc[:], in_=a2[e][:])
                nc.scalar.copy(os_sb[2 * p + e][:, ts(qt, QT_W)], a2[e][:])
                rh = p_rh.tile([DK + 1, QT_W], dt.float16, name="rh", tag="rh")
                nc.vector.tensor_copy(rh[DK:DK + 1, :], rc[DK:DK + 1, :])
                rhs.append(rh)
            return rhs

        def norm(h, qt, rh):
            """nt_pair[h] [:, qt] = os[h] * (1/denom) broadcast."""
            p, e = h // 2, h % 2
            bc = p_pt.tile([DK, QT_W], dt.float32, name="bc", tag="pt")
            nc.tensor.matmul(
                bc[:],
                ones[DK:DK + 1, :],
                rh[DK:DK + 1, :],
                start=True, stop=True,
            )
            nc.vector.tensor_mul(
                nt_sb[p][e * DK:(e + 1) * DK, ts(qt, QT_W)],
                os_sb[h][0:DK, ts(qt, QT_W)],
                bc[:],
            )

        # ================= schedule =================
        def F(*cs):
            return list(cs)

        def vnat_f(lo):
            return [lambda sc=sc: vnat(sc) for sc in range(lo, lo + 4)]

        def proj_f(st, pg):
            return [lambda: proj("k", st, pg), lambda: proj("q", st, pg)]

        def wo_f(st):
            return [lambda ec=ec: wo_unit(ec, st) for ec in range(NDC)]

        def norm_f(qt, rhs2, p):
            return [lambda e=e: norm(2 * p + e, qt, rhs2[e]) for e in (0, 1)]

        proj("k", 0, 0)
        proj("q", 0, 0)
        # W_O fillers are biased toward the late rounds: qt=3's exp load
        # saturates ScalarE, so the PE needs the most pad work there.
        rhs1 = None
        for qt in range(NQT):
            f0 = F(*proj_f(qt, 1), *vnat_f(4 * qt))
            if qt > 0:
                f0 += norm_f(qt - 1, rhs1, 1)
            rhs0 = attn_round(0, qt, f0)
            f1 = F(*(proj_f(qt + 1, 0) if qt < NQT - 1 else []))
            f1 += norm_f(qt, rhs0, 0)
            if qt == 1:
                f1 += wo_f(0)
            elif qt == 2:
                f1 += wo_f(1)[:5]
            elif qt == 3:
                # W_O fillers with no new dependencies come FIRST (qt=3 has
                # no proj fillers, and norm's bc would stall the PE queue on
                # the pair-0 recip/rh chain); then norm(3,p0), then pair-0's
                # half of W_O(3) which needs it.
                f1 = F(*wo_f(1)[5:], *norm_f(qt, rhs0, 0),
                       *(lambda ec=ec: wo_half(ec, 3, 0)
                         for ec in range(NDC)),
                       *wo_f(2)[:4])
            rhs1 = attn_round(1, qt, f1)
        # tail: reserved W_O(2) units pad the PE while the pair-1 recip/rh
        # chain for qt=3 drains. The scores PSUM pool is free now, so tail
        # units draw from it — their matmuls never wait on the backed-up
        # DVE queue via p_pt rotation. Casts alternate DVE/ScalarE.
        for i, ec in enumerate(range(4, NDC)):
            wo_unit(ec, NQT - 2, pool=p_s if i % 2 else None)
        for f in norm_f(NQT - 1, rhs1, 1):
            f()
        for ec in range(NDC):
            wo_half(ec, 3, 1, cast_eng=nc.scalar if ec % 2 else nc.vector,
                    pool=p_s if ec % 2 else None)

        if debug:
            nc.sync.dma_start(out=dbg["dbg_qt"][:], in_=qt_sb[0][:])
            nc.sync.dma_start(out=dbg["dbg_kt"][:], in_=kt_sb[0][:])
            nc.sync.dma_start(
                out=dbg["dbg_vp"][:],
                in_=vp_sb.rearrange("p h k w -> p (h k w)"))
            nc.sync.dma_start(out=dbg["dbg_os"][:], in_=os_sb[0][:])
            nc.sync.dma_start(out=dbg["dbg_nt"][:], in_=nt_sb[0][:])

    nc.compile()
    return nc


_NC = None


def _get_nc():
    global _NC
    if _NC is None:
        _NC = build()
    return _NC


def make_in_maps(x, W_Q, W_K, W_V, W_O):
    x = np.asarray(x, np.float32)
    W_Q, W_K, W_V, W_O = (np.asarray(w, np.float32)
                          for w in (W_Q, W_K, W_V, W_O))
    ones = np.ones((DK + 1, DK), np.float16)
    trim = (np.arange(KC_W)[:, None] <= np.arange(KC_W)[None, :]).astype(
        np.float16)
    def wlay(w):
        # [D, CW] -> [128, NDC, CW]: partition-contiguous DMA lines
        return np.ascontiguousarray(
            w.reshape(NDC, 128, CW).transpose(1, 0, 2)).astype(np.float16)

    in_maps = []
    for c in range(NCORES):
        b, g = c // HPC, c % HPC
        cols = slice(g * CW, (g + 1) * CW)
        # x[b].T is [D, S]; target [st][p][dc][j] with d = dc*128+p,
        # s = st*512+j
        xt = np.ascontiguousarray(
            x[b].T.reshape(NDC, 128, NQT, QT_W).transpose(2, 1, 0, 3)
        ).astype(np.float16)
        wo = np.ascontiguousarray(
            W_O[cols, :].reshape(2, 128, D).transpose(1, 0, 2)
        ).astype(np.float16)
        in_maps.append({
            "xt": xt,
            "wq": wlay(W_Q[:, cols]),
            "wk": wlay(W_K[:, cols]),
            "wv": wlay(W_V[:, cols]),
            "wo": wo,
            "ones": ones,
            "tri": trim,
        })
    return in_maps


def gather_output(results):
    out = np.zeros((B, S, D), np.float32)
    for c in range(NCORES):
        o = results[c]["o"].astype(np.float32)
        o[:, (NQT - 1) * QT_W:] += results[c]["o2"].astype(np.float32)
        out[c // HPC] += o.T
    return out


def kernel(x, W_Q, W_K, W_V, W_O):
    nc = _get_nc()
    res = run_bass_kernel_spmd(
        nc, make_in_maps(x, W_Q, W_K, W_V, W_O), list(range(NCORES))).results
    return gather_output(res)
